# revision 28
# baseline (speedup 1.0000x reference)
"""Trainium2 Bass kernel for nn_Attention2 (dense transformer attention).

Math (per batch element b):
  A = [enc_sum broadcast | z]          # [S, 320], enc part constant over S
  Q = A @ Wq ; K = A @ Wk ; V = A @ Wv
  scores = Q K^T / 16 ; mask query rows ; attn = softmax(scores)
  out = attn @ V

Key restructuring used on device (exact in math, better fp32 rounding):
  * scores rows are shifted by a row-constant (Q . k_enc) which softmax
    ignores -> scores~ = G @ z^T with G = Q @ Wk_z^T   (rank-64 contraction)
  * G^T = WG^T @ z^T + g_enc x 1 with WG = Wq_z @ Wk_z^T (64x64),
    g_enc = Wk_z @ q_enc^T
  * V is never materialized: out = rsum * (exp @ z) @ Wv_z + v_enc
  * query-row masking == zeroing G columns (masked rows -> uniform attn,
    bitwise-identical to the reference's -1e9 path)
  * scores~ in split-fp16 2-pass ("stacked" hi/lo) => ~fp32 accuracy at
    bf16 matmul speed.

Sharding: data-parallel over batch, B=8 -> one batch element per core.
"""

import numpy as np

S = 2048
H = 256
Z = 64
P = 128
NB = S // P  # 16 query blocks
TEMP_INV = 1.0 / 16.0

_CACHED = {}


def _build_nc():
    import concourse.bass as bass
    import concourse.tile as tile
    from concourse import bacc, mybir
    from concourse.masks import make_identity

    f32 = mybir.dt.float32
    f16 = mybir.dt.float16
    i32 = mybir.dt.int32
    AX = mybir.AxisListType
    OP = mybir.AluOpType
    AF = mybir.ActivationFunctionType

    nc = bacc.Bacc("TRN2", target_bir_lowering=False, debug=False)

    enc_t = nc.dram_tensor("enc", [2, H], f32, kind="ExternalInput")
    z_t = nc.dram_tensor("z", [S, Z], f32, kind="ExternalInput")
    mask_t = nc.dram_tensor("mask", [S], i32, kind="ExternalInput")
    wq_t = nc.dram_tensor("wq", [H + Z, H], f32, kind="ExternalInput")
    wk_t = nc.dram_tensor("wk", [H + Z, H], f32, kind="ExternalInput")
    wv_t = nc.dram_tensor("wv", [H + Z, H], f32, kind="ExternalInput")
    out_t = nc.dram_tensor("out", [S, H], f32, kind="ExternalOutput")
    attn_t = nc.dram_tensor("attn", [S, S], f32, kind="ExternalOutput")

    enc_ap = enc_t.ap()
    z_ap = z_t.ap()
    wq_ap = wq_t.ap()
    wk_ap = wk_t.ap()
    wv_ap = wv_t.ap()
    out_ap = out_t.ap()
    attn_ap = attn_t.ap()

    with tile.TileContext(nc) as tc:
        with tc.tile_pool(name="consts", bufs=1) as consts:
            ident = consts.tile([P, P], f32)
            make_identity(nc, ident)
            ident16 = consts.tile([P, P], f16)
            nc.vector.tensor_copy(ident16, ident)
            ones_row = consts.tile([1, 512], f32)
            nc.vector.memset(ones_row, 1.0)
            ones_1p = consts.tile([1, P], f32)
            nc.vector.memset(ones_1p, 1.0)

            # ---------------- input DMAs ----------------
            enc_sb = consts.tile([1, 2, H], f32)
            nc.sync.dma_start(
                out=enc_sb, in_=bass.AP(tensor=enc_t, offset=0, ap=[[0, 1], [H, 2], [1, H]])
            )
            wq_top = consts.tile([P, 2, H], f32)
            nc.sync.dma_start(out=wq_top, in_=wq_ap[0:H, :].rearrange("(c p) h -> p c h", p=P))
            wv_top = consts.tile([P, 2, H], f32)
            nc.sync.dma_start(out=wv_top, in_=wv_ap[0:H, :].rearrange("(c p) h -> p c h", p=P))
            wq_z = consts.tile([Z, H], f32)
            nc.sync.dma_start(out=wq_z, in_=wq_ap[H : H + Z, :])
            wk_z = consts.tile([Z, H], f32)
            nc.sync.dma_start(out=wk_z, in_=wk_ap[H : H + Z, :])
            wv_z = consts.tile([Z, H], f32)
            nc.sync.dma_start(out=wv_z, in_=wv_ap[H : H + Z, :])
            z_sb = consts.tile([P, NB, Z], f32)
            nc.sync.dma_start(out=z_sb, in_=z_ap.rearrange("(n p) d -> p n d", p=P))
            mask_bc = consts.tile([Z, S], f32)
            nc.gpsimd.dma_start(
                out=mask_bc, in_=bass.AP(tensor=mask_t, offset=0, ap=[[0, Z], [1, S]])
            )

            # ---------------- setup compute ----------------
            with tc.tile_pool(name="sps", bufs=6, space="PSUM") as sps:
                # z^T [Z, S] via PE transposes
                zT = consts.tile([Z, S], f32)
                for g in range(4):
                    zt_ps = sps.tile([P, 512], f32, tag="sps")
                    for u in range(4):
                        i = g * 4 + u
                        nc.tensor.transpose(zt_ps[0:Z, u * P : (u + 1) * P], z_sb[:, i, :], ident)
                    nc.vector.tensor_copy(zT[:, g * 512 : (g + 1) * 512], zt_ps[0:Z, :])

                # fp16 z variants
                z_f16 = consts.tile([P, NB, Z], f16)
                nc.vector.tensor_copy(z_f16, z_sb)
                zstk = consts.tile([P, S], f16)  # [zhi ; zhi] stacked
                nc.vector.tensor_copy(zstk[0:Z, :], zT)
                nc.vector.tensor_copy(zstk[Z : 2 * Z, :], zstk[0:Z, :])
                zloT = consts.tile([Z, S], f16)
                nc.vector.tensor_sub(zloT, zT, zstk[0:Z, :])

                # weight transposes: wq_zT, wk_zT  [P, 2, Z]
                wq_zT = consts.tile([P, 2, Z], f32)
                wk_zT = consts.tile([P, 2, Z], f32)
                for dst, src in ((wq_zT, wq_z), (wk_zT, wk_z)):
                    for c in range(2):
                        wt_ps = sps.tile([P, 512], f32, tag="sps")
                        nc.tensor.transpose(
                            wt_ps[:, 0:Z], src[:, c * P : (c + 1) * P], ident[0:Z, 0:Z]
                        )
                        nc.vector.tensor_copy(dst[:, c, :], wt_ps[:, 0:Z])

                # WG [Z, Z] = sum_c wq_zT[:,c,:].T @ wk_zT[:,c,:]
                WG = consts.tile([Z, Z], f32)
                wg_ps = sps.tile([P, 512], f32, tag="sps")
                for c in range(2):
                    nc.tensor.matmul(
                        wg_ps[0:Z, 0:Z], wq_zT[:, c, :], wk_zT[:, c, :],
                        start=(c == 0), stop=(c == 1),
                    )
                nc.vector.tensor_copy(WG, wg_ps[0:Z, 0:Z])

                # enc_sum [1, H] and its transpose [P, 2]
                enc_sum = consts.tile([1, H], f32)
                nc.vector.tensor_add(enc_sum, enc_sb[:, 0, :], enc_sb[:, 1, :])
                encT = consts.tile([P, 2], f32)
                for c in range(2):
                    et_ps = sps.tile([P, 512], f32, tag="sps")
                    nc.tensor.transpose(
                        et_ps[:, 0:1], enc_sum[:, c * P : (c + 1) * P], ident[0:1, 0:1]
                    )
                    nc.vector.tensor_copy(encT[:, c : c + 1], et_ps[:, 0:1])

                # q_enc, v_enc [1, H]
                q_enc = consts.tile([1, H], f32)
                v_enc = consts.tile([1, H], f32)
                for dst, wtop in ((q_enc, wq_top), (v_enc, wv_top)):
                    qe_ps = sps.tile([P, 512], f32, tag="sps")
                    for c in range(2):
                        nc.tensor.matmul(
                            qe_ps[0:1, 0:H], encT[:, c : c + 1], wtop[:, c, :],
                            start=(c == 0), stop=(c == 1),
                        )
                    nc.vector.tensor_copy(dst, qe_ps[0:1, 0:H])

                # q_encT [P, 2]
                q_encT = consts.tile([P, 2], f32)
                for c in range(2):
                    qt_ps = sps.tile([P, 512], f32, tag="sps")
                    nc.tensor.transpose(
                        qt_ps[:, 0:1], q_enc[:, c * P : (c + 1) * P], ident[0:1, 0:1]
                    )
                    nc.vector.tensor_copy(q_encT[:, c : c + 1], qt_ps[:, 0:1])

                # g_enc [1, Z] = q_enc @ wk_z^T
                g_enc = consts.tile([1, Z], f32)
                ge_ps = sps.tile([P, 512], f32, tag="sps")
                for c in range(2):
                    nc.tensor.matmul(
                        ge_ps[0:1, 0:Z], q_encT[:, c : c + 1], wk_zT[:, c, :],
                        start=(c == 0), stop=(c == 1),
                    )
                nc.vector.tensor_copy(g_enc, ge_ps[0:1, 0:Z])

                # v_enc broadcast to [P, H]
                v_enc_bc = consts.tile([P, H], f32)
                vb_ps = sps.tile([P, 512], f32, tag="sps")
                nc.tensor.matmul(vb_ps[:, 0:H], ones_1p, v_enc, start=True, stop=True)
                nc.vector.tensor_copy(v_enc_bc, vb_ps[:, 0:H])

                # G^T [Z, S] = WG^T-contraction @ zT + g_enc x ones, masked
                gT = consts.tile([Z, S], f32)
                for g in range(4):
                    gp = sps.tile([P, 512], f32, tag="sps")
                    sl = slice(g * 512, (g + 1) * 512)
                    nc.tensor.matmul(gp[0:Z, :], WG, zT[:, sl], start=True, stop=False)
                    nc.tensor.matmul(gp[0:Z, :], g_enc, ones_row, start=False, stop=True)
                    nc.vector.tensor_mul(gT[:, sl], gp[0:Z, :], mask_bc[:, sl])

                # G stacked fp16 split: [Ghi ; Glo]
                Gstk = consts.tile([P, S], f16)
                nc.vector.tensor_copy(Gstk[0:Z, :], gT)
                nc.vector.tensor_sub(Gstk[Z : 2 * Z, :], gT, Gstk[0:Z, :])

            # ---------------- main loop pools ----------------
            with (
                tc.tile_pool(name="ps_big", bufs=3, space="PSUM") as ps_big,
                tc.tile_pool(name="ps_tp", bufs=2, space="PSUM") as ps_tp,
                tc.tile_pool(name="scp", bufs=2) as scp,
                tc.tile_pool(name="expp", bufs=2) as expp,
                tc.tile_pool(name="expp16", bufs=2) as expp16,
                tc.tile_pool(name="attnp", bufs=2) as attnp,
                tc.tile_pool(name="expTp", bufs=3) as expTp,
                tc.tile_pool(name="smalls", bufs=6) as smalls,
                tc.tile_pool(name="rsums", bufs=6) as rsums,
                tc.tile_pool(name="tails", bufs=3) as tails,
                tc.tile_pool(name="outp", bufs=3) as outp,
            ):
                for qb in range(NB):
                    qsl = slice(qb * P, (qb + 1) * P)

                    # ---- scores~ = G-2pass @ z-2pass, two [P, 1024] pair-tiles
                    pairs = [
                        ps_big.tile([P, 1024], f32, tag="big", name=f"sc{qb}_{h}")
                        for h in range(2)
                    ]
                    for c in range(4):
                        ks = slice(c * 512, (c + 1) * 512)
                        osl = slice((c % 2) * 512, (c % 2) * 512 + 512)
                        nc.tensor.matmul(
                            pairs[c // 2][:, osl], Gstk[:, qsl], zstk[:, ks],
                            start=True, stop=False, skip_group_check=True,
                        )
                    for c in range(4):
                        ks = slice(c * 512, (c + 1) * 512)
                        osl = slice((c % 2) * 512, (c % 2) * 512 + 512)
                        nc.tensor.matmul(
                            pairs[c // 2][:, osl], Gstk[0:Z, qsl], zloT[:, ks],
                            start=False, stop=True, skip_group_check=True,
                        )

                    # ---- retire scores PSUM -> SBUF (scaled 1/16) with fused
                    # row-max accumulation (frees PSUM fast)
                    scsb = scp.tile([P, S], f32, tag="scsb")
                    rm = smalls.tile([P, 2], f32, tag="rm")
                    for h in range(2):
                        nc.vector.tensor_scalar(
                            scsb[:, h * 1024 : (h + 1) * 1024],
                            pairs[h],
                            TEMP_INV,
                            None,
                            op0=OP.mult,
                            op1=OP.max,
                            accum_out=rm[:, h : h + 1],
                        )
                    nbias16 = smalls.tile([P, 1], f32, tag="nbias16")
                    nc.vector.tensor_reduce(nbias16, rm, axis=AX.X, op=OP.max, negate=True)

                    # ---- exp pass 1 (fp16, feeds AV) + row sum via accum
                    exp16 = expp16.tile([P, S], f16, tag="exp16")
                    ssum = smalls.tile([P, 1], f32, tag="ssum")
                    nc.scalar.activation(
                        exp16, scsb, AF.Exp, bias=nbias16, scale=1.0, accum_out=ssum
                    )
                    rsum = smalls.tile([P, 1], f32, tag="rsum")
                    nc.vector.reciprocal(rsum, ssum)

                    # ---- exp pass 2: normalized f32 attn straight from scores
                    lnsum = smalls.tile([P, 1], f32, tag="lnsum")
                    nc.scalar.activation(lnsum, ssum, AF.Ln)
                    bias2 = smalls.tile([P, 1], f32, tag="bias2")
                    nc.vector.tensor_sub(bias2, nbias16, lnsum)
                    attn_sb = attnp.tile([P, S], f32, tag="attn")
                    nc.scalar.activation(attn_sb, scsb, AF.Exp, bias=bias2, scale=1.0)
                    nc.sync.dma_start(out=attn_ap[qsl, :], in_=attn_sb)

                    # ---- transpose exp16 -> expT [P, NB, P] (fp16, per block)
                    expT = expTp.tile([P, NB, P], f16, tag="expT")
                    for t in range(2):
                        tp = ps_tp.tile([P, 1024], f16, tag="tp")
                        for u in range(8):
                            kb = t * 8 + u
                            nc.tensor.transpose(
                                tp[:, u * P : (u + 1) * P], exp16[:, kb * P : (kb + 1) * P], ident16
                            )
                        src = tp.rearrange("p (a b) -> p a b", a=8)
                        if t == 0:
                            nc.scalar.copy(expT[:, 0:8, :], src)
                        else:
                            nc.vector.tensor_copy(expT[:, 8:16, :], src)

                    # ---- AV: attnz^T [Z, P] accumulated over k blocks
                    azT = ps_tp.tile([P, 256], f32, tag="tp", name=f"azT{qb}")
                    for kb in range(NB):
                        nc.tensor.matmul(
                            azT[0:Z, 0:P], z_f16[:, kb, :], expT[:, kb, :],
                            start=(kb == 0), stop=(kb == NB - 1),
                        )
                    azs = tails.tile([Z, P], f32, tag="azs")
                    nc.scalar.copy(azs, azT[0:Z, 0:P])
                    # out [q, h] = azs^T @ Wv_z (fp32), then normalize + v_enc
                    ob = ps_tp.tile([P, 512], f32, tag="tp", name=f"ob{qb}")
                    nc.tensor.matmul(ob[:, 0:H], azs, wv_z, start=True, stop=True)
                    ou = outp.tile([P, H], f32, tag="ou")
                    nc.vector.scalar_tensor_tensor(
                        out=ou, in0=ob[:, 0:H], scalar=rsum,
                        in1=v_enc_bc, op0=OP.mult, op1=OP.add,
                    )
                    nc.sync.dma_start(out=out_ap[qsl, :], in_=ou)

    nc.compile()
    return nc


_LDW_OPT = False
_ACT_PATCH = False


def _patch_act_tables():
    """Point walrus at an act_info.json with the natural_log_exp set listed
    first, so Exp and Ln resolve to ONE table set (no per-block
    ACT_TABLE_LOAD thrash)."""
    import json
    import os
    import tempfile

    if _CACHED.get("act_patched"):
        return
    try:
        from neuronxcc.driver.Job import Job
        from neuronxcc.driver.jobs.support.FindActInfo import findActInfoFile

        src = findActInfoFile(Job.getPackageDir(), "gen3")
        srcdir = os.path.dirname(src)
        d = json.load(open(src))
        sets = d["act_func_sets"]
        pref = [s for s in sets if s["name"] == "natural_log_exp_and_others"]
        rest = [s for s in sets if s["name"] != "natural_log_exp_and_others"]
        if not pref:
            return
        d["act_func_sets"] = pref + rest
        outdir = tempfile.mkdtemp(prefix="act_custom_")
        for fn in os.listdir(srcdir):
            if fn != "act_info.json":
                os.symlink(os.path.join(srcdir, fn), os.path.join(outdir, fn))
        with open(os.path.join(outdir, "act_info.json"), "w") as f:
            json.dump(d, f)
        os.environ["BASS_ACT_ROOT_JSON_PATH"] = os.path.join(outdir, "act_info.json")
        _CACHED["act_patched"] = True
    except Exception:
        pass


def _patch_ldw_opt():
    """Flip walrus's --enable-ldw-opt to true (dedups LDWEIGHTS / enables
    fast weight load). Done by rewriting the walrus argv at run_command."""
    if _CACHED.get("ldw_patched"):
        return
    from concourse import bass_utils as bu

    orig = bu.run_command

    def patched(argv, **kwargs):
        argv = [
            a.replace("--enable-ldw-opt=false", "--enable-ldw-opt=true")
            if isinstance(a, str)
            else a
            for a in argv
        ]
        return orig(argv, **kwargs)

    bu.run_command = patched
    _CACHED["ldw_patched"] = True


def _get_nc():
    if "nc" not in _CACHED:
        if _LDW_OPT:
            _patch_ldw_opt()
        if _ACT_PATCH:
            _patch_act_tables()
        _CACHED["nc"] = _build_nc()
    return _CACHED["nc"]


def kernel(
    encoder_hidden_state, decoder_hidden_state, latent_z_seq, mask, weight_q, weight_k, weight_v
):
    from concourse.bass_utils import run_bass_kernel_spmd

    B = latent_z_seq.shape[0]
    assert B == 8
    nc = _get_nc()

    enc = np.asarray(encoder_hidden_state, dtype=np.float32)
    z = np.asarray(latent_z_seq, dtype=np.float32)
    msk = np.asarray(mask, dtype=np.int32)
    wq = np.ascontiguousarray(np.asarray(weight_q, dtype=np.float32))
    wk = np.ascontiguousarray(np.asarray(weight_k, dtype=np.float32))
    wv = np.ascontiguousarray(np.asarray(weight_v, dtype=np.float32))

    in_maps = []
    for b in range(B):
        in_maps.append(
            {
                "enc": np.ascontiguousarray(enc[:, b, :]),
                "z": np.ascontiguousarray(z[b]),
                "mask": np.ascontiguousarray(msk[b]),
                "wq": wq,
                "wk": wk,
                "wv": wv,
            }
        )

    res = run_bass_kernel_spmd(nc, in_maps, core_ids=list(range(B)))
    _CACHED["last_results"] = res

    out = np.stack([res.results[b]["out"] for b in range(B)], axis=0)
    attn = np.stack([res.results[b]["attn"] for b in range(B)], axis=0)
    return (out, attn)


# revision 29
# speedup vs baseline: 1.3139x; 1.3139x over previous
"""Trainium2 Bass kernel for nn_Attention2 (dense transformer attention).

Math (per batch element b):
  A = [enc_sum broadcast | z]          # [S, 320], enc part constant over S
  Q = A @ Wq ; K = A @ Wk ; V = A @ Wv
  scores = Q K^T / 16 ; mask query rows ; attn = softmax(scores)
  out = attn @ V

Key restructuring used on device (exact in math, better fp32 rounding):
  * scores rows are shifted by a row-constant (Q . k_enc) which softmax
    ignores -> scores~ = G @ z^T with G = Q @ Wk_z^T   (rank-64 contraction)
  * G^T = WG^T @ z^T + g_enc x 1 with WG = Wq_z @ Wk_z^T (64x64),
    g_enc = Wk_z @ q_enc^T
  * V is never materialized: out = rsum * (exp @ z) @ Wv_z + v_enc
  * query-row masking == zeroing G columns (masked rows -> uniform attn,
    bitwise-identical to the reference's -1e9 path)
  * scores~ in split-fp16 2-pass ("stacked" hi/lo) => ~fp32 accuracy at
    bf16 matmul speed.

Sharding: data-parallel over batch, B=8 -> one batch element per core.
"""

import numpy as np

S = 2048
H = 256
Z = 64
P = 128
NB = S // P  # 16 query blocks
TEMP_INV = 1.0 / 16.0

_CACHED = {}


def _build_nc():
    import concourse.bass as bass
    import concourse.tile as tile
    from concourse import bacc, mybir
    from concourse.masks import make_identity

    f32 = mybir.dt.float32
    f16 = mybir.dt.float16
    i32 = mybir.dt.int32
    AX = mybir.AxisListType
    OP = mybir.AluOpType
    AF = mybir.ActivationFunctionType

    nc = bacc.Bacc("TRN2", target_bir_lowering=False, debug=False)

    enc_t = nc.dram_tensor("enc", [2, H], f32, kind="ExternalInput")
    z_t = nc.dram_tensor("z", [S, Z], f32, kind="ExternalInput")
    mask_t = nc.dram_tensor("mask", [S], i32, kind="ExternalInput")
    wq_t = nc.dram_tensor("wq", [H + Z, H], f32, kind="ExternalInput")
    wk_t = nc.dram_tensor("wk", [H + Z, H], f32, kind="ExternalInput")
    wv_t = nc.dram_tensor("wv", [H + Z, H], f32, kind="ExternalInput")
    out_t = nc.dram_tensor("out", [S, H], f32, kind="ExternalOutput")
    attn_t = nc.dram_tensor("attn", [S, S], f32, kind="ExternalOutput")

    enc_ap = enc_t.ap()
    z_ap = z_t.ap()
    wq_ap = wq_t.ap()
    wk_ap = wk_t.ap()
    wv_ap = wv_t.ap()
    out_ap = out_t.ap()
    attn_ap = attn_t.ap()

    with tile.TileContext(nc) as tc:
        with tc.tile_pool(name="consts", bufs=1) as consts:
            ident = consts.tile([P, P], f32)
            make_identity(nc, ident)
            ident16 = consts.tile([P, P], f16)
            nc.vector.tensor_copy(ident16, ident)
            ones_row = consts.tile([1, 512], f32)
            nc.vector.memset(ones_row, 1.0)
            ones_1p = consts.tile([1, P], f32)
            nc.vector.memset(ones_1p, 1.0)

            # ---------------- input DMAs ----------------
            enc_sb = consts.tile([1, 2, H], f32)
            nc.sync.dma_start(
                out=enc_sb, in_=bass.AP(tensor=enc_t, offset=0, ap=[[0, 1], [H, 2], [1, H]])
            )
            wq_top = consts.tile([P, 2, H], f32)
            nc.sync.dma_start(out=wq_top, in_=wq_ap[0:H, :].rearrange("(c p) h -> p c h", p=P))
            wv_top = consts.tile([P, 2, H], f32)
            nc.sync.dma_start(out=wv_top, in_=wv_ap[0:H, :].rearrange("(c p) h -> p c h", p=P))
            wq_z = consts.tile([Z, H], f32)
            nc.sync.dma_start(out=wq_z, in_=wq_ap[H : H + Z, :])
            wk_z = consts.tile([Z, H], f32)
            nc.sync.dma_start(out=wk_z, in_=wk_ap[H : H + Z, :])
            wv_z = consts.tile([Z, H], f32)
            nc.sync.dma_start(out=wv_z, in_=wv_ap[H : H + Z, :])
            z_sb = consts.tile([P, NB, Z], f32)
            nc.sync.dma_start(out=z_sb, in_=z_ap.rearrange("(n p) d -> p n d", p=P))
            mask_bc = consts.tile([Z, S], f32)
            nc.gpsimd.dma_start(
                out=mask_bc, in_=bass.AP(tensor=mask_t, offset=0, ap=[[0, Z], [1, S]])
            )

            # ---------------- setup compute ----------------
            with tc.tile_pool(name="sps", bufs=6, space="PSUM") as sps:
                # z^T [Z, S] via PE transposes
                zT = consts.tile([Z, S], f32)
                for g in range(4):
                    zt_ps = sps.tile([P, 512], f32, tag="sps")
                    for u in range(4):
                        i = g * 4 + u
                        nc.tensor.transpose(zt_ps[0:Z, u * P : (u + 1) * P], z_sb[:, i, :], ident)
                    nc.vector.tensor_copy(zT[:, g * 512 : (g + 1) * 512], zt_ps[0:Z, :])

                # fp16 z variants
                z_f16 = consts.tile([P, NB, Z], f16)
                nc.vector.tensor_copy(z_f16, z_sb)
                zstk = consts.tile([P, S], f16)  # [zhi ; zhi] stacked
                nc.vector.tensor_copy(zstk[0:Z, :], zT)
                nc.vector.tensor_copy(zstk[Z : 2 * Z, :], zstk[0:Z, :])
                zloT = consts.tile([Z, S], f16)
                nc.vector.tensor_sub(zloT, zT, zstk[0:Z, :])

                # weight transposes: wq_zT, wk_zT  [P, 2, Z]
                wq_zT = consts.tile([P, 2, Z], f32)
                wk_zT = consts.tile([P, 2, Z], f32)
                for dst, src in ((wq_zT, wq_z), (wk_zT, wk_z)):
                    for c in range(2):
                        wt_ps = sps.tile([P, 512], f32, tag="sps")
                        nc.tensor.transpose(
                            wt_ps[:, 0:Z], src[:, c * P : (c + 1) * P], ident[0:Z, 0:Z]
                        )
                        nc.vector.tensor_copy(dst[:, c, :], wt_ps[:, 0:Z])

                # WG [Z, Z] = sum_c wq_zT[:,c,:].T @ wk_zT[:,c,:]
                WG = consts.tile([Z, Z], f32)
                wg_ps = sps.tile([P, 512], f32, tag="sps")
                for c in range(2):
                    nc.tensor.matmul(
                        wg_ps[0:Z, 0:Z], wq_zT[:, c, :], wk_zT[:, c, :],
                        start=(c == 0), stop=(c == 1),
                    )
                nc.vector.tensor_copy(WG, wg_ps[0:Z, 0:Z])

                # enc_sum [1, H] and its transpose [P, 2]
                enc_sum = consts.tile([1, H], f32)
                nc.vector.tensor_add(enc_sum, enc_sb[:, 0, :], enc_sb[:, 1, :])
                encT = consts.tile([P, 2], f32)
                for c in range(2):
                    et_ps = sps.tile([P, 512], f32, tag="sps")
                    nc.tensor.transpose(
                        et_ps[:, 0:1], enc_sum[:, c * P : (c + 1) * P], ident[0:1, 0:1]
                    )
                    nc.vector.tensor_copy(encT[:, c : c + 1], et_ps[:, 0:1])

                # q_enc, v_enc [1, H]
                q_enc = consts.tile([1, H], f32)
                v_enc = consts.tile([1, H], f32)
                for dst, wtop in ((q_enc, wq_top), (v_enc, wv_top)):
                    qe_ps = sps.tile([P, 512], f32, tag="sps")
                    for c in range(2):
                        nc.tensor.matmul(
                            qe_ps[0:1, 0:H], encT[:, c : c + 1], wtop[:, c, :],
                            start=(c == 0), stop=(c == 1),
                        )
                    nc.vector.tensor_copy(dst, qe_ps[0:1, 0:H])

                # q_encT [P, 2]
                q_encT = consts.tile([P, 2], f32)
                for c in range(2):
                    qt_ps = sps.tile([P, 512], f32, tag="sps")
                    nc.tensor.transpose(
                        qt_ps[:, 0:1], q_enc[:, c * P : (c + 1) * P], ident[0:1, 0:1]
                    )
                    nc.vector.tensor_copy(q_encT[:, c : c + 1], qt_ps[:, 0:1])

                # g_enc [1, Z] = q_enc @ wk_z^T
                g_enc = consts.tile([1, Z], f32)
                ge_ps = sps.tile([P, 512], f32, tag="sps")
                for c in range(2):
                    nc.tensor.matmul(
                        ge_ps[0:1, 0:Z], q_encT[:, c : c + 1], wk_zT[:, c, :],
                        start=(c == 0), stop=(c == 1),
                    )
                nc.vector.tensor_copy(g_enc, ge_ps[0:1, 0:Z])

                # v_enc broadcast to [P, H]
                v_enc_bc = consts.tile([P, H], f32)
                vb_ps = sps.tile([P, 512], f32, tag="sps")
                nc.tensor.matmul(vb_ps[:, 0:H], ones_1p, v_enc, start=True, stop=True)
                nc.vector.tensor_copy(v_enc_bc, vb_ps[:, 0:H])

                # G^T [Z, S] = WG^T-contraction @ zT + g_enc x ones, masked
                gT = consts.tile([Z, S], f32)
                for g in range(4):
                    gp = sps.tile([P, 512], f32, tag="sps")
                    sl = slice(g * 512, (g + 1) * 512)
                    nc.tensor.matmul(gp[0:Z, :], WG, zT[:, sl], start=True, stop=False)
                    nc.tensor.matmul(gp[0:Z, :], g_enc, ones_row, start=False, stop=True)
                    nc.vector.tensor_mul(gT[:, sl], gp[0:Z, :], mask_bc[:, sl])

                # G stacked fp16 split: [Ghi ; Glo]
                Gstk = consts.tile([P, S], f16)
                nc.vector.tensor_copy(Gstk[0:Z, :], gT)
                nc.vector.tensor_sub(Gstk[Z : 2 * Z, :], gT, Gstk[0:Z, :])

            # ---------------- main loop pools ----------------
            with (
                tc.tile_pool(name="ps_big", bufs=3, space="PSUM") as ps_big,
                tc.tile_pool(name="ps_tp", bufs=2, space="PSUM") as ps_tp,
                tc.tile_pool(name="scp", bufs=2) as scp,
                tc.tile_pool(name="expp", bufs=2) as expp,
                tc.tile_pool(name="expp16", bufs=2) as expp16,
                tc.tile_pool(name="attnp", bufs=2) as attnp,
                tc.tile_pool(name="expTp", bufs=3) as expTp,
                tc.tile_pool(name="smalls", bufs=6) as smalls,
                tc.tile_pool(name="rsums", bufs=6) as rsums,
                tc.tile_pool(name="tails", bufs=3) as tails,
                tc.tile_pool(name="outp", bufs=3) as outp,
            ):
                for qb in range(NB):
                    qsl = slice(qb * P, (qb + 1) * P)

                    # ---- scores~ = G-2pass @ z-2pass, two [P, 1024] pair-tiles
                    pairs = [
                        ps_big.tile([P, 1024], f32, tag="big", name=f"sc{qb}_{h}")
                        for h in range(2)
                    ]
                    for c in range(4):
                        ks = slice(c * 512, (c + 1) * 512)
                        osl = slice((c % 2) * 512, (c % 2) * 512 + 512)
                        nc.tensor.matmul(
                            pairs[c // 2][:, osl], Gstk[:, qsl], zstk[:, ks],
                            start=True, stop=False, skip_group_check=True,
                        )
                    for c in range(4):
                        ks = slice(c * 512, (c + 1) * 512)
                        osl = slice((c % 2) * 512, (c % 2) * 512 + 512)
                        nc.tensor.matmul(
                            pairs[c // 2][:, osl], Gstk[0:Z, qsl], zloT[:, ks],
                            start=False, stop=True, skip_group_check=True,
                        )

                    # ---- retire scores PSUM -> SBUF (scaled 1/16) with fused
                    # row-max accumulation (frees PSUM fast)
                    scsb = scp.tile([P, S], f32, tag="scsb")
                    rm = smalls.tile([P, 2], f32, tag="rm")
                    for h in range(2):
                        nc.vector.tensor_scalar(
                            scsb[:, h * 1024 : (h + 1) * 1024],
                            pairs[h],
                            TEMP_INV,
                            None,
                            op0=OP.mult,
                            op1=OP.max,
                            accum_out=rm[:, h : h + 1],
                        )
                    nbias16 = smalls.tile([P, 1], f32, tag="nbias16")
                    nc.vector.tensor_reduce(nbias16, rm, axis=AX.X, op=OP.max, negate=True)

                    # ---- exp (f32) + row sum via accum (one big pass)
                    exp_sb = expp.tile([P, S], f32, tag="exp")
                    ssum = smalls.tile([P, 1], f32, tag="ssum")
                    nc.scalar.activation(
                        exp_sb, scsb, AF.Exp, bias=nbias16, scale=1.0, accum_out=ssum
                    )
                    rsum = smalls.tile([P, 1], f32, tag="rsum")
                    nc.vector.reciprocal(rsum, ssum)

                    # ---- normalized attn (DVE) -> HBM; fp16 exp copy (ACT)
                    attn_sb = attnp.tile([P, S], f32, tag="attn")
                    nc.vector.tensor_scalar_mul(attn_sb, exp_sb, rsum)
                    nc.sync.dma_start(out=attn_ap[qsl, :], in_=attn_sb)
                    exp16 = expp16.tile([P, S], f16, tag="exp16")
                    nc.scalar.copy(exp16, exp_sb)

                    # ---- transpose exp16 -> expT [P, NB, P] (fp16, per block)
                    expT = expTp.tile([P, NB, P], f16, tag="expT")
                    for t in range(2):
                        tp = ps_tp.tile([P, 1024], f16, tag="tp")
                        for u in range(8):
                            kb = t * 8 + u
                            nc.tensor.transpose(
                                tp[:, u * P : (u + 1) * P], exp16[:, kb * P : (kb + 1) * P], ident16
                            )
                        src = tp.rearrange("p (a b) -> p a b", a=8)
                        if t == 0:
                            nc.scalar.copy(expT[:, 0:8, :], src)
                        else:
                            nc.vector.tensor_copy(expT[:, 8:16, :], src)

                    # ---- AV: attnz^T [Z, P] accumulated over k blocks
                    azT = ps_tp.tile([P, 256], f32, tag="tp", name=f"azT{qb}")
                    for kb in range(NB):
                        nc.tensor.matmul(
                            azT[0:Z, 0:P], z_f16[:, kb, :], expT[:, kb, :],
                            start=(kb == 0), stop=(kb == NB - 1),
                        )
                    azs = tails.tile([Z, P], f32, tag="azs")
                    nc.scalar.copy(azs, azT[0:Z, 0:P])
                    # out [q, h] = azs^T @ Wv_z (fp32), then normalize + v_enc
                    ob = ps_tp.tile([P, 512], f32, tag="tp", name=f"ob{qb}")
                    nc.tensor.matmul(ob[:, 0:H], azs, wv_z, start=True, stop=True)
                    ou = outp.tile([P, H], f32, tag="ou")
                    nc.vector.scalar_tensor_tensor(
                        out=ou, in0=ob[:, 0:H], scalar=rsum,
                        in1=v_enc_bc, op0=OP.mult, op1=OP.add,
                    )
                    nc.sync.dma_start(out=out_ap[qsl, :], in_=ou)

    nc.compile()
    return nc


_LDW_OPT = False
_ACT_PATCH = False


def _patch_act_tables():
    """Point walrus at an act_info.json with the natural_log_exp set listed
    first, so Exp and Ln resolve to ONE table set (no per-block
    ACT_TABLE_LOAD thrash)."""
    import json
    import os
    import tempfile

    if _CACHED.get("act_patched"):
        return
    try:
        from neuronxcc.driver.Job import Job
        from neuronxcc.driver.jobs.support.FindActInfo import findActInfoFile

        src = findActInfoFile(Job.getPackageDir(), "gen3")
        srcdir = os.path.dirname(src)
        d = json.load(open(src))
        sets = d["act_func_sets"]
        pref = [s for s in sets if s["name"] == "natural_log_exp_and_others"]
        rest = [s for s in sets if s["name"] != "natural_log_exp_and_others"]
        if not pref:
            return
        d["act_func_sets"] = pref + rest
        outdir = tempfile.mkdtemp(prefix="act_custom_")
        for fn in os.listdir(srcdir):
            if fn != "act_info.json":
                os.symlink(os.path.join(srcdir, fn), os.path.join(outdir, fn))
        with open(os.path.join(outdir, "act_info.json"), "w") as f:
            json.dump(d, f)
        os.environ["BASS_ACT_ROOT_JSON_PATH"] = os.path.join(outdir, "act_info.json")
        _CACHED["act_patched"] = True
    except Exception:
        pass


def _patch_ldw_opt():
    """Flip walrus's --enable-ldw-opt to true (dedups LDWEIGHTS / enables
    fast weight load). Done by rewriting the walrus argv at run_command."""
    if _CACHED.get("ldw_patched"):
        return
    from concourse import bass_utils as bu

    orig = bu.run_command

    def patched(argv, **kwargs):
        argv = [
            a.replace("--enable-ldw-opt=false", "--enable-ldw-opt=true")
            if isinstance(a, str)
            else a
            for a in argv
        ]
        return orig(argv, **kwargs)

    bu.run_command = patched
    _CACHED["ldw_patched"] = True


def _get_nc():
    if "nc" not in _CACHED:
        if _LDW_OPT:
            _patch_ldw_opt()
        if _ACT_PATCH:
            _patch_act_tables()
        _CACHED["nc"] = _build_nc()
    return _CACHED["nc"]


def kernel(
    encoder_hidden_state, decoder_hidden_state, latent_z_seq, mask, weight_q, weight_k, weight_v
):
    from concourse.bass_utils import run_bass_kernel_spmd

    B = latent_z_seq.shape[0]
    assert B == 8
    nc = _get_nc()

    enc = np.asarray(encoder_hidden_state, dtype=np.float32)
    z = np.asarray(latent_z_seq, dtype=np.float32)
    msk = np.asarray(mask, dtype=np.int32)
    wq = np.ascontiguousarray(np.asarray(weight_q, dtype=np.float32))
    wk = np.ascontiguousarray(np.asarray(weight_k, dtype=np.float32))
    wv = np.ascontiguousarray(np.asarray(weight_v, dtype=np.float32))

    in_maps = []
    for b in range(B):
        in_maps.append(
            {
                "enc": np.ascontiguousarray(enc[:, b, :]),
                "z": np.ascontiguousarray(z[b]),
                "mask": np.ascontiguousarray(msk[b]),
                "wq": wq,
                "wk": wk,
                "wv": wv,
            }
        )

    res = run_bass_kernel_spmd(nc, in_maps, core_ids=list(range(B)))
    _CACHED["last_results"] = res

    out = np.stack([res.results[b]["out"] for b in range(B)], axis=0)
    attn = np.stack([res.results[b]["attn"] for b in range(B)], axis=0)
    return (out, attn)


# revision 30
# speedup vs baseline: 1.3546x; 1.0310x over previous
"""Trainium2 Bass kernel for nn_Attention2 (dense transformer attention).

Math (per batch element b):
  A = [enc_sum broadcast | z]          # [S, 320], enc part constant over S
  Q = A @ Wq ; K = A @ Wk ; V = A @ Wv
  scores = Q K^T / 16 ; mask query rows ; attn = softmax(scores)
  out = attn @ V

Key restructuring used on device (exact in math, better fp32 rounding):
  * scores rows are shifted by a row-constant (Q . k_enc) which softmax
    ignores -> scores~ = G @ z^T with G = Q @ Wk_z^T   (rank-64 contraction)
  * G^T = WG^T @ z^T + g_enc x 1 with WG = Wq_z @ Wk_z^T (64x64),
    g_enc = Wk_z @ q_enc^T
  * V is never materialized: out = rsum * (exp @ z) @ Wv_z + v_enc
  * query-row masking == zeroing G columns (masked rows -> uniform attn,
    bitwise-identical to the reference's -1e9 path)
  * scores~ in split-fp16 2-pass ("stacked" hi/lo) => ~fp32 accuracy at
    bf16 matmul speed.

Sharding: data-parallel over batch, B=8 -> one batch element per core.
"""

import numpy as np

S = 2048
H = 256
Z = 64
P = 128
NB = S // P  # 16 query blocks
TEMP_INV = 1.0 / 16.0

_CACHED = {}


def _build_nc():
    import concourse.bass as bass
    import concourse.tile as tile
    from concourse import bacc, mybir
    from concourse.masks import make_identity

    f32 = mybir.dt.float32
    f16 = mybir.dt.float16
    i32 = mybir.dt.int32
    AX = mybir.AxisListType
    OP = mybir.AluOpType
    AF = mybir.ActivationFunctionType

    nc = bacc.Bacc("TRN2", target_bir_lowering=False, debug=False)

    enc_t = nc.dram_tensor("enc", [2, H], f32, kind="ExternalInput")
    z_t = nc.dram_tensor("z", [S, Z], f32, kind="ExternalInput")
    mask_t = nc.dram_tensor("mask", [S], i32, kind="ExternalInput")
    wq_t = nc.dram_tensor("wq", [H + Z, H], f32, kind="ExternalInput")
    wk_t = nc.dram_tensor("wk", [H + Z, H], f32, kind="ExternalInput")
    wv_t = nc.dram_tensor("wv", [H + Z, H], f32, kind="ExternalInput")
    out_t = nc.dram_tensor("out", [S, H], f32, kind="ExternalOutput")
    attn_t = nc.dram_tensor("attn", [S, S], f32, kind="ExternalOutput")

    enc_ap = enc_t.ap()
    z_ap = z_t.ap()
    wq_ap = wq_t.ap()
    wk_ap = wk_t.ap()
    wv_ap = wv_t.ap()
    out_ap = out_t.ap()
    attn_ap = attn_t.ap()

    with tile.TileContext(nc) as tc:
        with tc.tile_pool(name="consts", bufs=1) as consts:
            ident = consts.tile([P, P], f32)
            make_identity(nc, ident)
            ident16 = consts.tile([P, P], f16)
            nc.vector.tensor_copy(ident16, ident)
            ones_row = consts.tile([1, 512], f32)
            nc.vector.memset(ones_row, 1.0)
            ones_1p = consts.tile([1, P], f32)
            nc.vector.memset(ones_1p, 1.0)

            # ---------------- input DMAs (z first: it gates PE setup) ----
            z_sb = consts.tile([P, NB, Z], f32)
            nc.sync.dma_start(out=z_sb, in_=z_ap.rearrange("(n p) d -> p n d", p=P))
            wk_z = consts.tile([Z, H], f32)
            nc.sync.dma_start(out=wk_z, in_=wk_ap[H : H + Z, :])
            wq_z = consts.tile([Z, H], f32)
            nc.sync.dma_start(out=wq_z, in_=wq_ap[H : H + Z, :])
            enc_sb = consts.tile([1, 2, H], f32)
            nc.sync.dma_start(
                out=enc_sb, in_=bass.AP(tensor=enc_t, offset=0, ap=[[0, 1], [H, 2], [1, H]])
            )
            wq_top = consts.tile([P, 2, H], f32)
            nc.sync.dma_start(out=wq_top, in_=wq_ap[0:H, :].rearrange("(c p) h -> p c h", p=P))
            wv_top = consts.tile([P, 2, H], f32)
            nc.sync.dma_start(out=wv_top, in_=wv_ap[0:H, :].rearrange("(c p) h -> p c h", p=P))
            wv_z = consts.tile([Z, H], f32)
            nc.sync.dma_start(out=wv_z, in_=wv_ap[H : H + Z, :])
            mask_bc = consts.tile([Z, S], f32)
            nc.gpsimd.dma_start(
                out=mask_bc, in_=bass.AP(tensor=mask_t, offset=0, ap=[[0, Z], [1, S]])
            )

            # ---------------- setup compute ----------------
            with tc.tile_pool(name="sps", bufs=6, space="PSUM") as sps:
                # z^T [Z, S] via PE transposes
                zT = consts.tile([Z, S], f32)
                for g in range(4):
                    zt_ps = sps.tile([P, 512], f32, tag="sps")
                    for u in range(4):
                        i = g * 4 + u
                        nc.tensor.transpose(zt_ps[0:Z, u * P : (u + 1) * P], z_sb[:, i, :], ident)
                    nc.vector.tensor_copy(zT[:, g * 512 : (g + 1) * 512], zt_ps[0:Z, :])

                # fp16 z variants
                z_f16 = consts.tile([P, NB, Z], f16)
                nc.vector.tensor_copy(z_f16, z_sb)
                zstk = consts.tile([P, S], f16)  # [zhi ; zhi] stacked
                nc.vector.tensor_copy(zstk[0:Z, :], zT)
                nc.vector.tensor_copy(zstk[Z : 2 * Z, :], zstk[0:Z, :])
                zloT = consts.tile([Z, S], f16)
                nc.vector.tensor_sub(zloT, zT, zstk[0:Z, :])

                # weight transposes: wq_zT, wk_zT  [P, 2, Z]
                wq_zT = consts.tile([P, 2, Z], f32)
                wk_zT = consts.tile([P, 2, Z], f32)
                for dst, src in ((wq_zT, wq_z), (wk_zT, wk_z)):
                    for c in range(2):
                        wt_ps = sps.tile([P, 512], f32, tag="sps")
                        nc.tensor.transpose(
                            wt_ps[:, 0:Z], src[:, c * P : (c + 1) * P], ident[0:Z, 0:Z]
                        )
                        nc.vector.tensor_copy(dst[:, c, :], wt_ps[:, 0:Z])

                # WG [Z, Z] = sum_c wq_zT[:,c,:].T @ wk_zT[:,c,:]
                WG = consts.tile([Z, Z], f32)
                wg_ps = sps.tile([P, 512], f32, tag="sps")
                for c in range(2):
                    nc.tensor.matmul(
                        wg_ps[0:Z, 0:Z], wq_zT[:, c, :], wk_zT[:, c, :],
                        start=(c == 0), stop=(c == 1),
                    )
                nc.vector.tensor_copy(WG, wg_ps[0:Z, 0:Z])

                # enc_sum [1, H] and its transpose [P, 2]
                enc_sum = consts.tile([1, H], f32)
                nc.vector.tensor_add(enc_sum, enc_sb[:, 0, :], enc_sb[:, 1, :])
                encT = consts.tile([P, 2], f32)
                for c in range(2):
                    et_ps = sps.tile([P, 512], f32, tag="sps")
                    nc.tensor.transpose(
                        et_ps[:, 0:1], enc_sum[:, c * P : (c + 1) * P], ident[0:1, 0:1]
                    )
                    nc.vector.tensor_copy(encT[:, c : c + 1], et_ps[:, 0:1])

                # q_enc, v_enc [1, H]
                q_enc = consts.tile([1, H], f32)
                v_enc = consts.tile([1, H], f32)
                for dst, wtop in ((q_enc, wq_top), (v_enc, wv_top)):
                    qe_ps = sps.tile([P, 512], f32, tag="sps")
                    for c in range(2):
                        nc.tensor.matmul(
                            qe_ps[0:1, 0:H], encT[:, c : c + 1], wtop[:, c, :],
                            start=(c == 0), stop=(c == 1),
                        )
                    nc.vector.tensor_copy(dst, qe_ps[0:1, 0:H])

                # q_encT [P, 2]
                q_encT = consts.tile([P, 2], f32)
                for c in range(2):
                    qt_ps = sps.tile([P, 512], f32, tag="sps")
                    nc.tensor.transpose(
                        qt_ps[:, 0:1], q_enc[:, c * P : (c + 1) * P], ident[0:1, 0:1]
                    )
                    nc.vector.tensor_copy(q_encT[:, c : c + 1], qt_ps[:, 0:1])

                # g_enc [1, Z] = q_enc @ wk_z^T
                g_enc = consts.tile([1, Z], f32)
                ge_ps = sps.tile([P, 512], f32, tag="sps")
                for c in range(2):
                    nc.tensor.matmul(
                        ge_ps[0:1, 0:Z], q_encT[:, c : c + 1], wk_zT[:, c, :],
                        start=(c == 0), stop=(c == 1),
                    )
                nc.vector.tensor_copy(g_enc, ge_ps[0:1, 0:Z])

                # v_enc broadcast to [P, H]
                v_enc_bc = consts.tile([P, H], f32)
                vb_ps = sps.tile([P, 512], f32, tag="sps")
                nc.tensor.matmul(vb_ps[:, 0:H], ones_1p, v_enc, start=True, stop=True)
                nc.vector.tensor_copy(v_enc_bc, vb_ps[:, 0:H])

                # G^T [Z, S] = WG^T-contraction @ zT + g_enc x ones, masked
                gT = consts.tile([Z, S], f32)
                for g in range(4):
                    gp = sps.tile([P, 512], f32, tag="sps")
                    sl = slice(g * 512, (g + 1) * 512)
                    nc.tensor.matmul(gp[0:Z, :], WG, zT[:, sl], start=True, stop=False)
                    nc.tensor.matmul(gp[0:Z, :], g_enc, ones_row, start=False, stop=True)
                    nc.vector.tensor_mul(gT[:, sl], gp[0:Z, :], mask_bc[:, sl])

                # G stacked fp16 split: [Ghi ; Glo] (per 512-chunk so early
                # query blocks unblock as soon as their G chunk is ready)
                Gstk = consts.tile([P, S], f16)
                Ghi2 = consts.tile([P, S], f16)  # [Ghi ; Ghi] for packed pass2
                for g in range(4):
                    sl = slice(g * 512, (g + 1) * 512)
                    nc.vector.tensor_copy(Gstk[0:Z, sl], gT[:, sl])
                    nc.vector.tensor_sub(Gstk[Z : 2 * Z, sl], gT[:, sl], Gstk[0:Z, sl])
                    nc.vector.tensor_copy(Ghi2[0:Z, sl], Gstk[0:Z, sl])
                    nc.vector.tensor_copy(Ghi2[Z : 2 * Z, sl], Gstk[0:Z, sl])
                zlo2 = consts.tile([P, S], f16)  # [zlo ; zlo]
                nc.vector.tensor_copy(zlo2[0:Z, :], zloT)
                nc.vector.tensor_copy(zlo2[Z : 2 * Z, :], zloT)

            # ---------------- main loop pools ----------------
            with (
                tc.tile_pool(name="ps_big", bufs=3, space="PSUM") as ps_big,
                tc.tile_pool(name="ps_tp", bufs=2, space="PSUM") as ps_tp,
                tc.tile_pool(name="scp", bufs=2) as scp,
                tc.tile_pool(name="expp", bufs=2) as expp,
                tc.tile_pool(name="expp16", bufs=2) as expp16,
                tc.tile_pool(name="attnp", bufs=2) as attnp,
                tc.tile_pool(name="expTp", bufs=3) as expTp,
                tc.tile_pool(name="smalls", bufs=6) as smalls,
                tc.tile_pool(name="rsums", bufs=6) as rsums,
                tc.tile_pool(name="tails", bufs=3) as tails,
                tc.tile_pool(name="outp", bufs=3) as outp,
            ):
                for qb in range(NB):
                    qsl = slice(qb * P, (qb + 1) * P)

                    # ---- scores~ = G-2pass @ z-2pass, two [P, 1024] pair-tiles
                    pairs = [
                        ps_big.tile([P, 1024], f32, tag="big", name=f"sc{qb}_{h}")
                        for h in range(2)
                    ]
                    for c in range(4):
                        ks = slice(c * 512, (c + 1) * 512)
                        osl = slice((c % 2) * 512, (c % 2) * 512 + 512)
                        nc.tensor.matmul(
                            pairs[c // 2][:, osl], Gstk[:, qsl], zstk[:, ks],
                            start=True, stop=False, skip_group_check=True,
                        )
                    for c in range(4):
                        ks = slice(c * 512, (c + 1) * 512)
                        osl = slice((c % 2) * 512, (c % 2) * 512 + 512)
                        rg = (c % 2) * Z  # row group 0 or 64: concurrent pairs
                        nc.tensor.matmul(
                            pairs[c // 2][:, osl],
                            Ghi2[rg : rg + Z, qsl],
                            zlo2[rg : rg + Z, ks],
                            start=False, stop=True, skip_group_check=True,
                            tile_position=(rg, 0),
                        )

                    # ---- retire scores PSUM -> SBUF (scaled 1/16) with fused
                    # row-max accumulation (frees PSUM fast)
                    scsb = scp.tile([P, S], f32, tag="scsb")
                    rm = smalls.tile([P, 2], f32, tag="rm")
                    for h in range(2):
                        nc.vector.tensor_scalar(
                            scsb[:, h * 1024 : (h + 1) * 1024],
                            pairs[h],
                            TEMP_INV,
                            None,
                            op0=OP.mult,
                            op1=OP.max,
                            accum_out=rm[:, h : h + 1],
                        )
                    nbias16 = smalls.tile([P, 1], f32, tag="nbias16")
                    nc.vector.tensor_reduce(nbias16, rm, axis=AX.X, op=OP.max, negate=True)

                    # ---- exp (f32) + row sum via accum (one big pass)
                    exp_sb = expp.tile([P, S], f32, tag="exp")
                    ssum = smalls.tile([P, 1], f32, tag="ssum")
                    nc.scalar.activation(
                        exp_sb, scsb, AF.Exp, bias=nbias16, scale=1.0, accum_out=ssum
                    )
                    rsum = smalls.tile([P, 1], f32, tag="rsum")
                    nc.vector.reciprocal(rsum, ssum)

                    # ---- normalized attn (DVE) -> HBM; fp16 exp copy (ACT)
                    attn_sb = attnp.tile([P, S], f32, tag="attn")
                    nc.vector.tensor_scalar_mul(attn_sb, exp_sb, rsum)
                    nc.sync.dma_start(out=attn_ap[qsl, :], in_=attn_sb)
                    exp16 = expp16.tile([P, S], f16, tag="exp16")
                    nc.scalar.copy(exp16, exp_sb)

                    # ---- transpose exp16 -> expT [P, NB, P] (fp16, per block)
                    expT = expTp.tile([P, NB, P], f16, tag="expT")
                    for t in range(2):
                        tp = ps_tp.tile([P, 1024], f16, tag="tp")
                        for u in range(8):
                            kb = t * 8 + u
                            nc.tensor.transpose(
                                tp[:, u * P : (u + 1) * P], exp16[:, kb * P : (kb + 1) * P], ident16
                            )
                        src = tp.rearrange("p (a b) -> p a b", a=8)
                        if t == 0:
                            nc.scalar.copy(expT[:, 0:8, :], src)
                        else:
                            nc.vector.tensor_copy(expT[:, 8:16, :], src)

                    # ---- AV: attnz^T [Z, P] accumulated over k blocks
                    azT = ps_tp.tile([P, 256], f32, tag="tp", name=f"azT{qb}")
                    for kb in range(NB):
                        nc.tensor.matmul(
                            azT[0:Z, 0:P], z_f16[:, kb, :], expT[:, kb, :],
                            start=(kb == 0), stop=(kb == NB - 1),
                        )
                    azs = tails.tile([Z, P], f32, tag="azs")
                    nc.scalar.copy(azs, azT[0:Z, 0:P])
                    # out [q, h] = azs^T @ Wv_z (fp32), then normalize + v_enc
                    ob = ps_tp.tile([P, 512], f32, tag="tp", name=f"ob{qb}")
                    nc.tensor.matmul(ob[:, 0:H], azs, wv_z, start=True, stop=True)
                    ou = outp.tile([P, H], f32, tag="ou")
                    nc.vector.scalar_tensor_tensor(
                        out=ou, in0=ob[:, 0:H], scalar=rsum,
                        in1=v_enc_bc, op0=OP.mult, op1=OP.add,
                    )
                    nc.sync.dma_start(out=out_ap[qsl, :], in_=ou)

    nc.compile()
    return nc


_LDW_OPT = False
_ACT_PATCH = False


def _patch_act_tables():
    """Point walrus at an act_info.json with the natural_log_exp set listed
    first, so Exp and Ln resolve to ONE table set (no per-block
    ACT_TABLE_LOAD thrash)."""
    import json
    import os
    import tempfile

    if _CACHED.get("act_patched"):
        return
    try:
        from neuronxcc.driver.Job import Job
        from neuronxcc.driver.jobs.support.FindActInfo import findActInfoFile

        src = findActInfoFile(Job.getPackageDir(), "gen3")
        srcdir = os.path.dirname(src)
        d = json.load(open(src))
        sets = d["act_func_sets"]
        pref = [s for s in sets if s["name"] == "natural_log_exp_and_others"]
        rest = [s for s in sets if s["name"] != "natural_log_exp_and_others"]
        if not pref:
            return
        d["act_func_sets"] = pref + rest
        outdir = tempfile.mkdtemp(prefix="act_custom_")
        for fn in os.listdir(srcdir):
            if fn != "act_info.json":
                os.symlink(os.path.join(srcdir, fn), os.path.join(outdir, fn))
        with open(os.path.join(outdir, "act_info.json"), "w") as f:
            json.dump(d, f)
        os.environ["BASS_ACT_ROOT_JSON_PATH"] = os.path.join(outdir, "act_info.json")
        _CACHED["act_patched"] = True
    except Exception:
        pass


def _patch_ldw_opt():
    """Flip walrus's --enable-ldw-opt to true (dedups LDWEIGHTS / enables
    fast weight load). Done by rewriting the walrus argv at run_command."""
    if _CACHED.get("ldw_patched"):
        return
    from concourse import bass_utils as bu

    orig = bu.run_command

    def patched(argv, **kwargs):
        argv = [
            a.replace("--enable-ldw-opt=false", "--enable-ldw-opt=true")
            if isinstance(a, str)
            else a
            for a in argv
        ]
        return orig(argv, **kwargs)

    bu.run_command = patched
    _CACHED["ldw_patched"] = True


def _get_nc():
    if "nc" not in _CACHED:
        if _LDW_OPT:
            _patch_ldw_opt()
        if _ACT_PATCH:
            _patch_act_tables()
        _CACHED["nc"] = _build_nc()
    return _CACHED["nc"]


def kernel(
    encoder_hidden_state, decoder_hidden_state, latent_z_seq, mask, weight_q, weight_k, weight_v
):
    from concourse.bass_utils import run_bass_kernel_spmd

    B = latent_z_seq.shape[0]
    assert B == 8
    nc = _get_nc()

    enc = np.asarray(encoder_hidden_state, dtype=np.float32)
    z = np.asarray(latent_z_seq, dtype=np.float32)
    msk = np.asarray(mask, dtype=np.int32)
    wq = np.ascontiguousarray(np.asarray(weight_q, dtype=np.float32))
    wk = np.ascontiguousarray(np.asarray(weight_k, dtype=np.float32))
    wv = np.ascontiguousarray(np.asarray(weight_v, dtype=np.float32))

    in_maps = []
    for b in range(B):
        in_maps.append(
            {
                "enc": np.ascontiguousarray(enc[:, b, :]),
                "z": np.ascontiguousarray(z[b]),
                "mask": np.ascontiguousarray(msk[b]),
                "wq": wq,
                "wk": wk,
                "wv": wv,
            }
        )

    res = run_bass_kernel_spmd(nc, in_maps, core_ids=list(range(B)))
    _CACHED["last_results"] = res

    out = np.stack([res.results[b]["out"] for b in range(B)], axis=0)
    attn = np.stack([res.results[b]["attn"] for b in range(B)], axis=0)
    return (out, attn)


# revision 31
# speedup vs baseline: 1.3863x; 1.0235x over previous
"""Trainium2 Bass kernel for nn_Attention2 (dense transformer attention).

Math (per batch element b):
  A = [enc_sum broadcast | z]          # [S, 320], enc part constant over S
  Q = A @ Wq ; K = A @ Wk ; V = A @ Wv
  scores = Q K^T / 16 ; mask query rows ; attn = softmax(scores)
  out = attn @ V

Key restructuring used on device (exact in math, better fp32 rounding):
  * scores rows are shifted by a row-constant (Q . k_enc) which softmax
    ignores -> scores~ = G @ z^T with G = Q @ Wk_z^T   (rank-64 contraction)
  * G^T = WG^T @ z^T + g_enc x 1 with WG = Wq_z @ Wk_z^T (64x64),
    g_enc = Wk_z @ q_enc^T
  * V is never materialized: out = rsum * (exp @ z) @ Wv_z + v_enc
  * query-row masking == zeroing G columns (masked rows -> uniform attn,
    bitwise-identical to the reference's -1e9 path)
  * scores~ in split-fp16 2-pass ("stacked" hi/lo) => ~fp32 accuracy at
    bf16 matmul speed.

Sharding: data-parallel over batch, B=8 -> one batch element per core.
"""

import numpy as np

S = 2048
H = 256
Z = 64
P = 128
NB = S // P  # 16 query blocks
TEMP_INV = 1.0 / 16.0

_CACHED = {}


def _build_nc():
    import concourse.bass as bass
    import concourse.tile as tile
    from concourse import bacc, mybir
    from concourse.masks import make_identity

    f32 = mybir.dt.float32
    f16 = mybir.dt.float16
    i32 = mybir.dt.int32
    AX = mybir.AxisListType
    OP = mybir.AluOpType
    AF = mybir.ActivationFunctionType

    nc = bacc.Bacc("TRN2", target_bir_lowering=False, debug=False)

    enc_t = nc.dram_tensor("enc", [2, H], f32, kind="ExternalInput")
    z_t = nc.dram_tensor("z", [S, Z], f32, kind="ExternalInput")
    mask_t = nc.dram_tensor("mask", [S], i32, kind="ExternalInput")
    wq_t = nc.dram_tensor("wq", [H + Z, H], f32, kind="ExternalInput")
    wk_t = nc.dram_tensor("wk", [H + Z, H], f32, kind="ExternalInput")
    wv_t = nc.dram_tensor("wv", [H + Z, H], f32, kind="ExternalInput")
    out_t = nc.dram_tensor("out", [S, H], f32, kind="ExternalOutput")
    attn_t = nc.dram_tensor("attn", [S, S], f32, kind="ExternalOutput")

    enc_ap = enc_t.ap()
    z_ap = z_t.ap()
    wq_ap = wq_t.ap()
    wk_ap = wk_t.ap()
    wv_ap = wv_t.ap()
    out_ap = out_t.ap()
    attn_ap = attn_t.ap()

    with tile.TileContext(nc) as tc:
        with tc.tile_pool(name="consts", bufs=1) as consts:
            ident = consts.tile([P, P], f32)
            make_identity(nc, ident)
            ident16 = consts.tile([P, P], f16)
            nc.vector.tensor_copy(ident16, ident)
            ones_row = consts.tile([1, 512], f32)
            nc.vector.memset(ones_row, 1.0)
            ones_1p = consts.tile([1, P], f32)
            nc.vector.memset(ones_1p, 1.0)

            # ---------------- input DMAs (z first: it gates PE setup) ----
            z_sb = consts.tile([P, NB, Z], f32)
            nc.sync.dma_start(out=z_sb, in_=z_ap.rearrange("(n p) d -> p n d", p=P))
            wk_z = consts.tile([Z, H], f32)
            nc.sync.dma_start(out=wk_z, in_=wk_ap[H : H + Z, :])
            wq_z = consts.tile([Z, H], f32)
            nc.sync.dma_start(out=wq_z, in_=wq_ap[H : H + Z, :])
            enc_sb = consts.tile([1, 2, H], f32)
            nc.sync.dma_start(
                out=enc_sb, in_=bass.AP(tensor=enc_t, offset=0, ap=[[0, 1], [H, 2], [1, H]])
            )
            wq_top = consts.tile([P, 2, H], f32)
            nc.sync.dma_start(out=wq_top, in_=wq_ap[0:H, :].rearrange("(c p) h -> p c h", p=P))
            wv_top = consts.tile([P, 2, H], f32)
            nc.sync.dma_start(out=wv_top, in_=wv_ap[0:H, :].rearrange("(c p) h -> p c h", p=P))
            wv_z = consts.tile([Z, H], f32)
            nc.sync.dma_start(out=wv_z, in_=wv_ap[H : H + Z, :])
            mask_bc = consts.tile([Z, S], f32)
            nc.gpsimd.dma_start(
                out=mask_bc, in_=bass.AP(tensor=mask_t, offset=0, ap=[[0, Z], [1, S]])
            )

            # ---------------- setup compute ----------------
            with tc.tile_pool(name="sps", bufs=6, space="PSUM") as sps:
                # --- enc chain first (longest dependency chain) ---
                enc_sum = consts.tile([1, H], f32)
                nc.vector.tensor_add(enc_sum, enc_sb[:, 0, :], enc_sb[:, 1, :])
                encT = consts.tile([P, 2], f32)
                for c in range(2):
                    et_ps = sps.tile([P, 512], f32, tag="sps")
                    nc.tensor.transpose(
                        et_ps[:, 0:1], enc_sum[:, c * P : (c + 1) * P], ident[0:1, 0:1]
                    )
                    nc.vector.tensor_copy(encT[:, c : c + 1], et_ps[:, 0:1])

                # weight transposes: wq_zT, wk_zT  [P, 2, Z]
                wq_zT = consts.tile([P, 2, Z], f32)
                wk_zT = consts.tile([P, 2, Z], f32)
                for dst, srcw in ((wq_zT, wq_z), (wk_zT, wk_z)):
                    for c in range(2):
                        wt_ps = sps.tile([P, 512], f32, tag="sps")
                        nc.tensor.transpose(
                            wt_ps[:, 0:Z], srcw[:, c * P : (c + 1) * P], ident[0:Z, 0:Z]
                        )
                        nc.vector.tensor_copy(dst[:, c, :], wt_ps[:, 0:Z])

                # q_encT [P, 2] directly: chunk h of q_enc^T = sum_c wq_top^T-MM
                q_encT = consts.tile([P, 2], f32)
                qt_ps = sps.tile([P, 512], f32, tag="sps")
                for hc in range(2):
                    for c in range(2):
                        nc.tensor.matmul(
                            qt_ps[:, hc : hc + 1],
                            wq_top[:, c, hc * P : (hc + 1) * P],
                            encT[:, c : c + 1],
                            start=(c == 0), stop=(c == 1),
                        )
                nc.vector.tensor_copy(q_encT, qt_ps[:, 0:2])

                # g_enc [1, Z] = q_enc @ wk_z^T
                g_enc = consts.tile([1, Z], f32)
                ge_ps = sps.tile([P, 512], f32, tag="sps")
                for c in range(2):
                    nc.tensor.matmul(
                        ge_ps[0:1, 0:Z], q_encT[:, c : c + 1], wk_zT[:, c, :],
                        start=(c == 0), stop=(c == 1),
                    )
                nc.vector.tensor_copy(g_enc, ge_ps[0:1, 0:Z])

                # WG [Z, Z] = sum_c wq_zT[:,c,:].T @ wk_zT[:,c,:]
                WG = consts.tile([Z, Z], f32)
                wg_ps = sps.tile([P, 512], f32, tag="sps")
                for c in range(2):
                    nc.tensor.matmul(
                        wg_ps[0:Z, 0:Z], wq_zT[:, c, :], wk_zT[:, c, :],
                        start=(c == 0), stop=(c == 1),
                    )
                nc.vector.tensor_copy(WG, wg_ps[0:Z, 0:Z])

                # z^T [Z, S] via PE transposes (fills PE gaps of the chain)
                zT = consts.tile([Z, S], f32)
                for g in range(4):
                    zt_ps = sps.tile([P, 512], f32, tag="sps")
                    for u in range(4):
                        i = g * 4 + u
                        nc.tensor.transpose(zt_ps[0:Z, u * P : (u + 1) * P], z_sb[:, i, :], ident)
                    nc.vector.tensor_copy(zT[:, g * 512 : (g + 1) * 512], zt_ps[0:Z, :])

                # fp16 z variants
                z_f16 = consts.tile([P, NB, Z], f16)
                nc.vector.tensor_copy(z_f16, z_sb)
                zstk = consts.tile([P, S], f16)  # [zhi ; zhi] stacked
                nc.vector.tensor_copy(zstk[0:Z, :], zT)
                nc.vector.tensor_copy(zstk[Z : 2 * Z, :], zstk[0:Z, :])
                zloT = consts.tile([Z, S], f16)
                nc.vector.tensor_sub(zloT, zT, zstk[0:Z, :])
                zlo2 = consts.tile([P, S], f16)  # [zlo ; zlo]
                nc.vector.tensor_copy(zlo2[0:Z, :], zloT)
                nc.vector.tensor_copy(zlo2[Z : 2 * Z, :], zloT)

                # G^T [Z, S] = WG-contraction @ zT + g_enc x ones, masked,
                # then split hi/lo per 512-chunk (early blocks unblock early)
                gT = consts.tile([Z, S], f32)
                Gstk = consts.tile([P, S], f16)
                Ghi2 = consts.tile([P, S], f16)  # [Ghi ; Ghi] for packed pass2
                for g in range(4):
                    gp = sps.tile([P, 512], f32, tag="sps")
                    sl = slice(g * 512, (g + 1) * 512)
                    nc.tensor.matmul(gp[0:Z, :], WG, zT[:, sl], start=True, stop=False)
                    nc.tensor.matmul(gp[0:Z, :], g_enc, ones_row, start=False, stop=True)
                    nc.vector.tensor_mul(gT[:, sl], gp[0:Z, :], mask_bc[:, sl])
                    nc.vector.tensor_copy(Gstk[0:Z, sl], gT[:, sl])
                    nc.vector.tensor_sub(Gstk[Z : 2 * Z, sl], gT[:, sl], Gstk[0:Z, sl])
                    nc.vector.tensor_copy(Ghi2[0:Z, sl], Gstk[0:Z, sl])
                    nc.vector.tensor_copy(Ghi2[Z : 2 * Z, sl], Gstk[0:Z, sl])

                # v_enc path (only needed by the out-stage, off critical path)
                v_enc = consts.tile([1, H], f32)
                qe_ps = sps.tile([P, 512], f32, tag="sps")
                for c in range(2):
                    nc.tensor.matmul(
                        qe_ps[0:1, 0:H], encT[:, c : c + 1], wv_top[:, c, :],
                        start=(c == 0), stop=(c == 1),
                    )
                nc.vector.tensor_copy(v_enc, qe_ps[0:1, 0:H])
                v_enc_bc = consts.tile([P, H], f32)
                vb_ps = sps.tile([P, 512], f32, tag="sps")
                nc.tensor.matmul(vb_ps[:, 0:H], ones_1p, v_enc, start=True, stop=True)
                nc.vector.tensor_copy(v_enc_bc, vb_ps[:, 0:H])

            # ---------------- main loop pools ----------------
            with (
                tc.tile_pool(name="ps_big", bufs=3, space="PSUM") as ps_big,
                tc.tile_pool(name="ps_tp", bufs=2, space="PSUM") as ps_tp,
                tc.tile_pool(name="scp", bufs=2) as scp,
                tc.tile_pool(name="expp", bufs=2) as expp,
                tc.tile_pool(name="expp16", bufs=2) as expp16,
                tc.tile_pool(name="attnp", bufs=2) as attnp,
                tc.tile_pool(name="expTp", bufs=3) as expTp,
                tc.tile_pool(name="smalls", bufs=6) as smalls,
                tc.tile_pool(name="rsums", bufs=6) as rsums,
                tc.tile_pool(name="tails", bufs=3) as tails,
                tc.tile_pool(name="outp", bufs=3) as outp,
            ):
                for qb in range(NB):
                    qsl = slice(qb * P, (qb + 1) * P)

                    # ---- scores~ = G-2pass @ z-2pass, two [P, 1024] pair-tiles
                    pairs = [
                        ps_big.tile([P, 1024], f32, tag="big", name=f"sc{qb}_{h}")
                        for h in range(2)
                    ]
                    for c in range(4):
                        ks = slice(c * 512, (c + 1) * 512)
                        osl = slice((c % 2) * 512, (c % 2) * 512 + 512)
                        nc.tensor.matmul(
                            pairs[c // 2][:, osl], Gstk[:, qsl], zstk[:, ks],
                            start=True, stop=False, skip_group_check=True,
                        )
                    for c in range(4):
                        ks = slice(c * 512, (c + 1) * 512)
                        osl = slice((c % 2) * 512, (c % 2) * 512 + 512)
                        rg = (c % 2) * Z  # row group 0 or 64: concurrent pairs
                        nc.tensor.matmul(
                            pairs[c // 2][:, osl],
                            Ghi2[rg : rg + Z, qsl],
                            zlo2[rg : rg + Z, ks],
                            start=False, stop=True, skip_group_check=True,
                            tile_position=(rg, 0),
                        )

                    # ---- retire scores PSUM -> SBUF (scaled 1/16) with fused
                    # row-max accumulation (frees PSUM fast)
                    scsb = scp.tile([P, S], f32, tag="scsb")
                    rm = smalls.tile([P, 2], f32, tag="rm")
                    for h in range(2):
                        nc.vector.tensor_scalar(
                            scsb[:, h * 1024 : (h + 1) * 1024],
                            pairs[h],
                            TEMP_INV,
                            None,
                            op0=OP.mult,
                            op1=OP.max,
                            accum_out=rm[:, h : h + 1],
                        )
                    nbias16 = smalls.tile([P, 1], f32, tag="nbias16")
                    nc.vector.tensor_reduce(nbias16, rm, axis=AX.X, op=OP.max, negate=True)

                    # ---- exp (f32) + row sum via accum (one big pass)
                    exp_sb = expp.tile([P, S], f32, tag="exp")
                    ssum = smalls.tile([P, 1], f32, tag="ssum")
                    nc.scalar.activation(
                        exp_sb, scsb, AF.Exp, bias=nbias16, scale=1.0, accum_out=ssum
                    )
                    rsum = smalls.tile([P, 1], f32, tag="rsum")
                    nc.vector.reciprocal(rsum, ssum)

                    # ---- normalized attn (DVE) -> HBM; fp16 exp copy (ACT)
                    attn_sb = attnp.tile([P, S], f32, tag="attn")
                    nc.vector.tensor_scalar_mul(attn_sb, exp_sb, rsum)
                    nc.sync.dma_start(out=attn_ap[qsl, :], in_=attn_sb)
                    exp16 = expp16.tile([P, S], f16, tag="exp16")
                    nc.scalar.copy(exp16, exp_sb)

                    # ---- transpose exp16 -> expT [P, NB, P] (fp16, per block)
                    expT = expTp.tile([P, NB, P], f16, tag="expT")
                    for t in range(2):
                        tp = ps_tp.tile([P, 1024], f16, tag="tp")
                        for u in range(8):
                            kb = t * 8 + u
                            nc.tensor.transpose(
                                tp[:, u * P : (u + 1) * P], exp16[:, kb * P : (kb + 1) * P], ident16
                            )
                        src = tp.rearrange("p (a b) -> p a b", a=8)
                        if t == 0:
                            nc.scalar.copy(expT[:, 0:8, :], src)
                        else:
                            nc.vector.tensor_copy(expT[:, 8:16, :], src)

                    # ---- AV: attnz^T [Z, P] accumulated over k blocks
                    azT = ps_tp.tile([P, 256], f32, tag="tp", name=f"azT{qb}")
                    for kb in range(NB):
                        nc.tensor.matmul(
                            azT[0:Z, 0:P], z_f16[:, kb, :], expT[:, kb, :],
                            start=(kb == 0), stop=(kb == NB - 1),
                        )
                    azs = tails.tile([Z, P], f32, tag="azs")
                    nc.scalar.copy(azs, azT[0:Z, 0:P])
                    # out [q, h] = azs^T @ Wv_z (fp32), then normalize + v_enc
                    ob = ps_tp.tile([P, 512], f32, tag="tp", name=f"ob{qb}")
                    nc.tensor.matmul(ob[:, 0:H], azs, wv_z, start=True, stop=True)
                    ou = outp.tile([P, H], f32, tag="ou")
                    nc.vector.scalar_tensor_tensor(
                        out=ou, in0=ob[:, 0:H], scalar=rsum,
                        in1=v_enc_bc, op0=OP.mult, op1=OP.add,
                    )
                    nc.sync.dma_start(out=out_ap[qsl, :], in_=ou)

    nc.compile()
    return nc


_LDW_OPT = False
_ACT_PATCH = False


def _patch_act_tables():
    """Point walrus at an act_info.json with the natural_log_exp set listed
    first, so Exp and Ln resolve to ONE table set (no per-block
    ACT_TABLE_LOAD thrash)."""
    import json
    import os
    import tempfile

    if _CACHED.get("act_patched"):
        return
    try:
        from neuronxcc.driver.Job import Job
        from neuronxcc.driver.jobs.support.FindActInfo import findActInfoFile

        src = findActInfoFile(Job.getPackageDir(), "gen3")
        srcdir = os.path.dirname(src)
        d = json.load(open(src))
        sets = d["act_func_sets"]
        pref = [s for s in sets if s["name"] == "natural_log_exp_and_others"]
        rest = [s for s in sets if s["name"] != "natural_log_exp_and_others"]
        if not pref:
            return
        d["act_func_sets"] = pref + rest
        outdir = tempfile.mkdtemp(prefix="act_custom_")
        for fn in os.listdir(srcdir):
            if fn != "act_info.json":
                os.symlink(os.path.join(srcdir, fn), os.path.join(outdir, fn))
        with open(os.path.join(outdir, "act_info.json"), "w") as f:
            json.dump(d, f)
        os.environ["BASS_ACT_ROOT_JSON_PATH"] = os.path.join(outdir, "act_info.json")
        _CACHED["act_patched"] = True
    except Exception:
        pass


def _patch_ldw_opt():
    """Flip walrus's --enable-ldw-opt to true (dedups LDWEIGHTS / enables
    fast weight load). Done by rewriting the walrus argv at run_command."""
    if _CACHED.get("ldw_patched"):
        return
    from concourse import bass_utils as bu

    orig = bu.run_command

    def patched(argv, **kwargs):
        argv = [
            a.replace("--enable-ldw-opt=false", "--enable-ldw-opt=true")
            if isinstance(a, str)
            else a
            for a in argv
        ]
        return orig(argv, **kwargs)

    bu.run_command = patched
    _CACHED["ldw_patched"] = True


def _get_nc():
    if "nc" not in _CACHED:
        if _LDW_OPT:
            _patch_ldw_opt()
        if _ACT_PATCH:
            _patch_act_tables()
        _CACHED["nc"] = _build_nc()
    return _CACHED["nc"]


def kernel(
    encoder_hidden_state, decoder_hidden_state, latent_z_seq, mask, weight_q, weight_k, weight_v
):
    from concourse.bass_utils import run_bass_kernel_spmd

    B = latent_z_seq.shape[0]
    assert B == 8
    nc = _get_nc()

    enc = np.asarray(encoder_hidden_state, dtype=np.float32)
    z = np.asarray(latent_z_seq, dtype=np.float32)
    msk = np.asarray(mask, dtype=np.int32)
    wq = np.ascontiguousarray(np.asarray(weight_q, dtype=np.float32))
    wk = np.ascontiguousarray(np.asarray(weight_k, dtype=np.float32))
    wv = np.ascontiguousarray(np.asarray(weight_v, dtype=np.float32))

    in_maps = []
    for b in range(B):
        in_maps.append(
            {
                "enc": np.ascontiguousarray(enc[:, b, :]),
                "z": np.ascontiguousarray(z[b]),
                "mask": np.ascontiguousarray(msk[b]),
                "wq": wq,
                "wk": wk,
                "wv": wv,
            }
        )

    res = run_bass_kernel_spmd(nc, in_maps, core_ids=list(range(B)))
    _CACHED["last_results"] = res

    out = np.stack([res.results[b]["out"] for b in range(B)], axis=0)
    attn = np.stack([res.results[b]["attn"] for b in range(B)], axis=0)
    return (out, attn)


# revision 32
# speedup vs baseline: 1.4493x; 1.0454x over previous
"""Trainium2 Bass kernel for nn_Attention2 (dense transformer attention).

Math (per batch element b):
  A = [enc_sum broadcast | z]          # [S, 320], enc part constant over S
  Q = A @ Wq ; K = A @ Wk ; V = A @ Wv
  scores = Q K^T / 16 ; mask query rows ; attn = softmax(scores)
  out = attn @ V

Key restructuring used on device (exact in math, better fp32 rounding):
  * scores rows are shifted by a row-constant (Q . k_enc) which softmax
    ignores -> scores~ = G @ z^T with G = Q @ Wk_z^T   (rank-64 contraction)
  * G^T = WG^T @ z^T + g_enc x 1 with WG = Wq_z @ Wk_z^T (64x64),
    g_enc = Wk_z @ q_enc^T
  * V is never materialized: out = rsum * (exp @ z) @ Wv_z + v_enc
  * query-row masking == zeroing G columns (masked rows -> uniform attn,
    bitwise-identical to the reference's -1e9 path)
  * scores~ in split-fp16 2-pass ("stacked" hi/lo) => ~fp32 accuracy at
    bf16 matmul speed.

Sharding: data-parallel over batch, B=8 -> one batch element per core.
"""

import numpy as np

S = 2048
H = 256
Z = 64
P = 128
NB = S // P  # 16 query blocks
TEMP_INV = 1.0 / 16.0

_CACHED = {}


def _build_nc():
    import concourse.bass as bass
    import concourse.tile as tile
    from concourse import bacc, mybir
    from concourse.masks import make_identity

    f32 = mybir.dt.float32
    f16 = mybir.dt.float16
    bf16 = mybir.dt.bfloat16
    i32 = mybir.dt.int32
    AX = mybir.AxisListType
    OP = mybir.AluOpType
    AF = mybir.ActivationFunctionType

    nc = bacc.Bacc("TRN2", target_bir_lowering=False, debug=False)

    enc_t = nc.dram_tensor("enc", [2, H], f32, kind="ExternalInput")
    z_t = nc.dram_tensor("z", [S, Z], f32, kind="ExternalInput")
    mask_t = nc.dram_tensor("mask", [S], i32, kind="ExternalInput")
    wq_t = nc.dram_tensor("wq", [H + Z, H], f32, kind="ExternalInput")
    wk_t = nc.dram_tensor("wk", [H + Z, H], f32, kind="ExternalInput")
    wv_t = nc.dram_tensor("wv", [H + Z, H], f32, kind="ExternalInput")
    out_t = nc.dram_tensor("out", [S, H], f32, kind="ExternalOutput")
    attn_t = nc.dram_tensor("attn", [S, S], f32, kind="ExternalOutput")

    enc_ap = enc_t.ap()
    z_ap = z_t.ap()
    wq_ap = wq_t.ap()
    wk_ap = wk_t.ap()
    wv_ap = wv_t.ap()
    out_ap = out_t.ap()
    attn_ap = attn_t.ap()

    with tile.TileContext(nc) as tc:
        with tc.tile_pool(name="consts", bufs=1) as consts:
            ident = consts.tile([P, P], f32)
            make_identity(nc, ident)
            ident16 = consts.tile([P, P], f16)
            nc.vector.tensor_copy(ident16, ident)
            identB = consts.tile([P, P], bf16)
            nc.vector.tensor_copy(identB, ident)
            ones_row = consts.tile([1, 512], f32)
            nc.vector.memset(ones_row, 1.0)
            ones_1p = consts.tile([1, P], f32)
            nc.vector.memset(ones_1p, 1.0)

            # ---------------- input DMAs (z first: it gates PE setup) ----
            z_sb = consts.tile([P, NB, Z], f32)
            nc.sync.dma_start(out=z_sb, in_=z_ap.rearrange("(n p) d -> p n d", p=P))
            wk_z = consts.tile([Z, H], f32)
            nc.sync.dma_start(out=wk_z, in_=wk_ap[H : H + Z, :])
            wq_z = consts.tile([Z, H], f32)
            nc.sync.dma_start(out=wq_z, in_=wq_ap[H : H + Z, :])
            enc_sb = consts.tile([1, 2, H], f32)
            nc.sync.dma_start(
                out=enc_sb, in_=bass.AP(tensor=enc_t, offset=0, ap=[[0, 1], [H, 2], [1, H]])
            )
            wq_top = consts.tile([P, 2, H], f32)
            nc.sync.dma_start(out=wq_top, in_=wq_ap[0:H, :].rearrange("(c p) h -> p c h", p=P))
            wv_top = consts.tile([P, 2, H], f32)
            nc.sync.dma_start(out=wv_top, in_=wv_ap[0:H, :].rearrange("(c p) h -> p c h", p=P))
            wv_z = consts.tile([Z, H], f32)
            nc.sync.dma_start(out=wv_z, in_=wv_ap[H : H + Z, :])
            mask_bc = consts.tile([Z, S], f32)
            nc.gpsimd.dma_start(
                out=mask_bc, in_=bass.AP(tensor=mask_t, offset=0, ap=[[0, Z], [1, S]])
            )

            # ---------------- setup compute ----------------
            with tc.tile_pool(name="sps", bufs=6, space="PSUM") as sps:
                # --- enc chain first (longest dependency chain) ---
                enc_sum = consts.tile([1, H], f32)
                nc.vector.tensor_add(enc_sum, enc_sb[:, 0, :], enc_sb[:, 1, :])
                encT = consts.tile([P, 2], f32)
                for c in range(2):
                    et_ps = sps.tile([P, 512], f32, tag="sps")
                    nc.tensor.transpose(
                        et_ps[:, 0:1], enc_sum[:, c * P : (c + 1) * P], ident[0:1, 0:1]
                    )
                    nc.vector.tensor_copy(encT[:, c : c + 1], et_ps[:, 0:1])

                # weight transposes: wq_zT, wk_zT  [P, 2, Z]
                wq_zT = consts.tile([P, 2, Z], f32)
                wk_zT = consts.tile([P, 2, Z], f32)
                for dst, srcw in ((wq_zT, wq_z), (wk_zT, wk_z)):
                    for c in range(2):
                        wt_ps = sps.tile([P, 512], f32, tag="sps")
                        nc.tensor.transpose(
                            wt_ps[:, 0:Z], srcw[:, c * P : (c + 1) * P], ident[0:Z, 0:Z]
                        )
                        nc.vector.tensor_copy(dst[:, c, :], wt_ps[:, 0:Z])

                # q_encT [P, 2] directly: chunk h of q_enc^T = sum_c wq_top^T-MM
                q_encT = consts.tile([P, 2], f32)
                qt_ps = sps.tile([P, 512], f32, tag="sps")
                for hc in range(2):
                    for c in range(2):
                        nc.tensor.matmul(
                            qt_ps[:, hc : hc + 1],
                            wq_top[:, c, hc * P : (hc + 1) * P],
                            encT[:, c : c + 1],
                            start=(c == 0), stop=(c == 1),
                        )
                nc.vector.tensor_copy(q_encT, qt_ps[:, 0:2])

                # g_enc [1, Z] = q_enc @ wk_z^T
                g_enc = consts.tile([1, Z], f32)
                ge_ps = sps.tile([P, 512], f32, tag="sps")
                for c in range(2):
                    nc.tensor.matmul(
                        ge_ps[0:1, 0:Z], q_encT[:, c : c + 1], wk_zT[:, c, :],
                        start=(c == 0), stop=(c == 1),
                    )
                nc.vector.tensor_copy(g_enc, ge_ps[0:1, 0:Z])

                # WG [Z, Z] = sum_c wq_zT[:,c,:].T @ wk_zT[:,c,:]
                WG = consts.tile([Z, Z], f32)
                wg_ps = sps.tile([P, 512], f32, tag="sps")
                for c in range(2):
                    nc.tensor.matmul(
                        wg_ps[0:Z, 0:Z], wq_zT[:, c, :], wk_zT[:, c, :],
                        start=(c == 0), stop=(c == 1),
                    )
                nc.vector.tensor_copy(WG, wg_ps[0:Z, 0:Z])

                # z^T [Z, S] via PE transposes (fills PE gaps of the chain)
                zT = consts.tile([Z, S], f32)
                for g in range(4):
                    zt_ps = sps.tile([P, 512], f32, tag="sps")
                    for u in range(4):
                        i = g * 4 + u
                        nc.tensor.transpose(zt_ps[0:Z, u * P : (u + 1) * P], z_sb[:, i, :], ident)
                    nc.vector.tensor_copy(zT[:, g * 512 : (g + 1) * 512], zt_ps[0:Z, :])

                # fp16 z variants
                z_bf16 = consts.tile([P, NB, Z], bf16)
                nc.vector.tensor_copy(z_bf16, z_sb)
                zstk = consts.tile([P, S], f16)  # [zhi ; zhi] stacked
                nc.vector.tensor_copy(zstk[0:Z, :], zT)
                nc.vector.tensor_copy(zstk[Z : 2 * Z, :], zstk[0:Z, :])
                zloT = consts.tile([Z, S], f16)
                nc.vector.tensor_sub(zloT, zT, zstk[0:Z, :])
                zlo2 = consts.tile([P, S], f16)  # [zlo ; zlo]
                nc.vector.tensor_copy(zlo2[0:Z, :], zloT)
                nc.vector.tensor_copy(zlo2[Z : 2 * Z, :], zloT)

                # G^T [Z, S] = WG-contraction @ zT + g_enc x ones, masked,
                # then split hi/lo per 512-chunk (early blocks unblock early)
                gT = consts.tile([Z, S], f32)
                Gstk = consts.tile([P, S], f16)
                Ghi2 = consts.tile([P, S], f16)  # [Ghi ; Ghi] for packed pass2
                for g in range(4):
                    gp = sps.tile([P, 512], f32, tag="sps")
                    sl = slice(g * 512, (g + 1) * 512)
                    nc.tensor.matmul(gp[0:Z, :], WG, zT[:, sl], start=True, stop=False)
                    nc.tensor.matmul(gp[0:Z, :], g_enc, ones_row, start=False, stop=True)
                    nc.vector.tensor_mul(gT[:, sl], gp[0:Z, :], mask_bc[:, sl])
                    nc.vector.tensor_copy(Gstk[0:Z, sl], gT[:, sl])
                    nc.vector.tensor_sub(Gstk[Z : 2 * Z, sl], gT[:, sl], Gstk[0:Z, sl])
                    nc.vector.tensor_copy(Ghi2[0:Z, sl], Gstk[0:Z, sl])
                    nc.vector.tensor_copy(Ghi2[Z : 2 * Z, sl], Gstk[0:Z, sl])

                # v_enc path (only needed by the out-stage, off critical path)
                v_enc = consts.tile([1, H], f32)
                qe_ps = sps.tile([P, 512], f32, tag="sps")
                for c in range(2):
                    nc.tensor.matmul(
                        qe_ps[0:1, 0:H], encT[:, c : c + 1], wv_top[:, c, :],
                        start=(c == 0), stop=(c == 1),
                    )
                nc.vector.tensor_copy(v_enc, qe_ps[0:1, 0:H])
                v_enc_bc = consts.tile([P, H], f32)
                vb_ps = sps.tile([P, 512], f32, tag="sps")
                nc.tensor.matmul(vb_ps[:, 0:H], ones_1p, v_enc, start=True, stop=True)
                nc.vector.tensor_copy(v_enc_bc, vb_ps[:, 0:H])

            # ---------------- main loop pools ----------------
            with (
                tc.tile_pool(name="ps_big", bufs=3, space="PSUM") as ps_big,
                tc.tile_pool(name="ps_tp", bufs=2, space="PSUM") as ps_tp,
                tc.tile_pool(name="scp", bufs=2) as scp,
                tc.tile_pool(name="expp", bufs=2) as expp,
                tc.tile_pool(name="attnp", bufs=2) as attnp,
                tc.tile_pool(name="expTp", bufs=3) as expTp,
                tc.tile_pool(name="smalls", bufs=6) as smalls,
                tc.tile_pool(name="rsums", bufs=6) as rsums,
                tc.tile_pool(name="tails", bufs=3) as tails,
                tc.tile_pool(name="outp", bufs=3) as outp,
            ):
                for qb in range(NB):
                    qsl = slice(qb * P, (qb + 1) * P)

                    # ---- scores~ = G-2pass @ z-2pass, two [P, 1024] pair-tiles
                    pairs = [
                        ps_big.tile([P, 1024], f32, tag="big", name=f"sc{qb}_{h}")
                        for h in range(2)
                    ]
                    for c in range(4):
                        ks = slice(c * 512, (c + 1) * 512)
                        osl = slice((c % 2) * 512, (c % 2) * 512 + 512)
                        nc.tensor.matmul(
                            pairs[c // 2][:, osl], Gstk[:, qsl], zstk[:, ks],
                            start=True, stop=False, skip_group_check=True,
                        )
                    for c in range(4):
                        ks = slice(c * 512, (c + 1) * 512)
                        osl = slice((c % 2) * 512, (c % 2) * 512 + 512)
                        rg = (c % 2) * Z  # row group 0 or 64: concurrent pairs
                        nc.tensor.matmul(
                            pairs[c // 2][:, osl],
                            Ghi2[rg : rg + Z, qsl],
                            zlo2[rg : rg + Z, ks],
                            start=False, stop=True, skip_group_check=True,
                            tile_position=(rg, 0),
                        )

                    # ---- retire scores PSUM -> SBUF (scaled 1/16) with fused
                    # row-max accumulation (frees PSUM fast)
                    scsb = scp.tile([P, S], f32, tag="scsb")
                    rm = smalls.tile([P, 2], f32, tag="rm")
                    for h in range(2):
                        nc.vector.tensor_scalar(
                            scsb[:, h * 1024 : (h + 1) * 1024],
                            pairs[h],
                            TEMP_INV,
                            None,
                            op0=OP.mult,
                            op1=OP.max,
                            accum_out=rm[:, h : h + 1],
                        )
                    nbias16 = smalls.tile([P, 1], f32, tag="nbias16")
                    nc.vector.tensor_reduce(nbias16, rm, axis=AX.X, op=OP.max, negate=True)

                    # ---- exp (f32) + row sum via accum (one big pass)
                    exp_sb = expp.tile([P, S], f32, tag="exp")
                    ssum = smalls.tile([P, 1], f32, tag="ssum")
                    nc.scalar.activation(
                        exp_sb, scsb, AF.Exp, bias=nbias16, scale=1.0, accum_out=ssum
                    )
                    rsum = smalls.tile([P, 1], f32, tag="rsum")
                    nc.vector.reciprocal(rsum, ssum)

                    # ---- normalized attn (DVE) -> HBM; fp16 exp copy (ACT)
                    attn_sb = attnp.tile([P, S], f32, tag="attn")
                    nc.vector.tensor_scalar_mul(attn_sb[:, 0:1024], exp_sb[:, 0:1024], rsum)
                    nc.scalar.activation(
                        attn_sb[:, 1024:2048], exp_sb[:, 1024:2048], AF.Copy,
                        bias=0.0, scale=rsum,
                    )
                    nc.sync.dma_start(out=attn_ap[qsl, :], in_=attn_sb)
                    # ---- transpose high-half (bf16) view of exp -> expT
                    expbf = exp_sb.bitcast(bf16)
                    expT = expTp.tile([P, NB, P], bf16, tag="expT")
                    for t in range(2):
                        tp = ps_tp.tile([P, 1024], bf16, tag="tp")
                        for u in range(8):
                            kb = t * 8 + u
                            nc.tensor.transpose(
                                tp[:, u * P : (u + 1) * P],
                                expbf[:, kb * 256 + 1 : (kb + 1) * 256 : 2],
                                identB,
                            )
                        src = tp.rearrange("p (a b) -> p a b", a=8)
                        nc.scalar.copy(expT[:, t * 8 : (t + 1) * 8, :], src)

                    # ---- AV: attnz^T [Z, P] accumulated over k blocks
                    azT = ps_tp.tile([P, 256], f32, tag="tp", name=f"azT{qb}")
                    for kb in range(NB):
                        nc.tensor.matmul(
                            azT[0:Z, 0:P], z_bf16[:, kb, :], expT[:, kb, :],
                            start=(kb == 0), stop=(kb == NB - 1),
                        )
                    azs = tails.tile([Z, P], f32, tag="azs")
                    nc.scalar.copy(azs, azT[0:Z, 0:P])
                    # out [q, h] = azs^T @ Wv_z (fp32), then normalize + v_enc
                    ob = ps_tp.tile([P, 512], f32, tag="tp", name=f"ob{qb}")
                    nc.tensor.matmul(ob[:, 0:H], azs, wv_z, start=True, stop=True)
                    ou = outp.tile([P, H], f32, tag="ou")
                    nc.vector.scalar_tensor_tensor(
                        out=ou, in0=ob[:, 0:H], scalar=rsum,
                        in1=v_enc_bc, op0=OP.mult, op1=OP.add,
                    )
                    nc.sync.dma_start(out=out_ap[qsl, :], in_=ou)

    nc.compile()
    return nc


_LDW_OPT = False
_ACT_PATCH = False


def _patch_act_tables():
    """Point walrus at an act_info.json with the natural_log_exp set listed
    first, so Exp and Ln resolve to ONE table set (no per-block
    ACT_TABLE_LOAD thrash)."""
    import json
    import os
    import tempfile

    if _CACHED.get("act_patched"):
        return
    try:
        from neuronxcc.driver.Job import Job
        from neuronxcc.driver.jobs.support.FindActInfo import findActInfoFile

        src = findActInfoFile(Job.getPackageDir(), "gen3")
        srcdir = os.path.dirname(src)
        d = json.load(open(src))
        sets = d["act_func_sets"]
        pref = [s for s in sets if s["name"] == "natural_log_exp_and_others"]
        rest = [s for s in sets if s["name"] != "natural_log_exp_and_others"]
        if not pref:
            return
        d["act_func_sets"] = pref + rest
        outdir = tempfile.mkdtemp(prefix="act_custom_")
        for fn in os.listdir(srcdir):
            if fn != "act_info.json":
                os.symlink(os.path.join(srcdir, fn), os.path.join(outdir, fn))
        with open(os.path.join(outdir, "act_info.json"), "w") as f:
            json.dump(d, f)
        os.environ["BASS_ACT_ROOT_JSON_PATH"] = os.path.join(outdir, "act_info.json")
        _CACHED["act_patched"] = True
    except Exception:
        pass


def _patch_ldw_opt():
    """Flip walrus's --enable-ldw-opt to true (dedups LDWEIGHTS / enables
    fast weight load). Done by rewriting the walrus argv at run_command."""
    if _CACHED.get("ldw_patched"):
        return
    from concourse import bass_utils as bu

    orig = bu.run_command

    def patched(argv, **kwargs):
        argv = [
            a.replace("--enable-ldw-opt=false", "--enable-ldw-opt=true")
            if isinstance(a, str)
            else a
            for a in argv
        ]
        return orig(argv, **kwargs)

    bu.run_command = patched
    _CACHED["ldw_patched"] = True


def _get_nc():
    if "nc" not in _CACHED:
        if _LDW_OPT:
            _patch_ldw_opt()
        if _ACT_PATCH:
            _patch_act_tables()
        _CACHED["nc"] = _build_nc()
    return _CACHED["nc"]


def kernel(
    encoder_hidden_state, decoder_hidden_state, latent_z_seq, mask, weight_q, weight_k, weight_v
):
    from concourse.bass_utils import run_bass_kernel_spmd

    B = latent_z_seq.shape[0]
    assert B == 8
    nc = _get_nc()

    enc = np.asarray(encoder_hidden_state, dtype=np.float32)
    z = np.asarray(latent_z_seq, dtype=np.float32)
    msk = np.asarray(mask, dtype=np.int32)
    wq = np.ascontiguousarray(np.asarray(weight_q, dtype=np.float32))
    wk = np.ascontiguousarray(np.asarray(weight_k, dtype=np.float32))
    wv = np.ascontiguousarray(np.asarray(weight_v, dtype=np.float32))

    in_maps = []
    for b in range(B):
        in_maps.append(
            {
                "enc": np.ascontiguousarray(enc[:, b, :]),
                "z": np.ascontiguousarray(z[b]),
                "mask": np.ascontiguousarray(msk[b]),
                "wq": wq,
                "wk": wk,
                "wv": wv,
            }
        )

    res = run_bass_kernel_spmd(nc, in_maps, core_ids=list(range(B)))
    _CACHED["last_results"] = res

    out = np.stack([res.results[b]["out"] for b in range(B)], axis=0)
    attn = np.stack([res.results[b]["attn"] for b in range(B)], axis=0)
    return (out, attn)
